# revision 1
# baseline (speedup 1.0000x reference)
"""Causal MHSA (pre-LN, relative position bias, residual) on 8 Trainium2 cores.

Sharding: batch (4) x head-half (2) -> 8 cores. Core c handles batch c//2 and
heads (c%2)*8 .. (c%2)*8+8. Each core computes LN + Q/K/V projections for its
512 head-dims, causal attention for its 8 heads, and a partial output
projection. Host sums the two per-batch partials and adds the residual.

Math layout per core (T=2048, D=1024, dh=64):
  xcsT[d, t]  = ((x - mu) * rstd)^T           (fp16, via DMA transpose)
  qT/kT[m, t] = (W~^T as lhsT) @ xcsT          (gamma, 1/sqrt(dh) folded in W~)
  v[t, m]     = xcsT as lhsT @ wvT
  S[i, j]     = qT_slice.T @ kT   (+rel bias via exp bias / E-mask, causal)
  P = exp(S + rel128) (Z via accum_out), mask-add -30000 above diag first
  P *= E near-diag band; P *= 1/Z;  PT = dma-transpose(P)
  U1T[m, i]   = v_slice.T @ PT  (accumulated over j tiles) -> yT
  out[t, d]   = yT as lhsT @ woT  (partial; host adds pair + residual)
"""

import math
import sys

sys.path.insert(0, "/opt/trn_rl_repo")

import numpy as np
from contextlib import ExitStack

import concourse.bacc as bacc
import concourse.tile as tile
import concourse.mybir as mybir
from concourse.bass_utils import run_bass_kernel_spmd

F32 = mybir.dt.float32
F16 = mybir.dt.float16

T = 2048
D = 1024
DH = 64
NH = 8  # heads per core
M = NH * DH  # 512 head-dims per core
TT = T // 128  # 16 token tiles
DT = D // 128  # 8 d-chunks
MT = M // 128  # 4 m-tiles
NCORES = 8
LN_EPS = 1e-5
MASK_NEG = -30000.0

_CACHED_NC = None


def build_nc():
    nc = bacc.Bacc("TRN2", target_bir_lowering=False, debug=False, num_devices=NCORES)

    x_d = nc.dram_tensor("x", [T, D], F32, kind="ExternalInput")
    wqT_d = nc.dram_tensor("wqT", [D, M], F16, kind="ExternalInput")
    wkT_d = nc.dram_tensor("wkT", [D, M], F16, kind="ExternalInput")
    wvT_d = nc.dram_tensor("wvT", [D, M], F16, kind="ExternalInput")
    woT_d = nc.dram_tensor("woT", [M, D], F16, kind="ExternalInput")
    bmask_d = nc.dram_tensor("bmask", [128, NH * 256], F16, kind="ExternalInput")
    rel128_d = nc.dram_tensor("rel128", [128, NH], F32, kind="ExternalInput")
    bq_d = nc.dram_tensor("bq", [128, MT], F32, kind="ExternalInput")
    bk_d = nc.dram_tensor("bk", [128, MT], F32, kind="ExternalInput")
    bv_d = nc.dram_tensor("bv", [128, M], F16, kind="ExternalInput")
    out_d = nc.dram_tensor("out", [T, D], F32, kind="ExternalOutput")

    with tile.TileContext(nc) as tc, ExitStack() as ctx:
        singles = ctx.enter_context(tc.tile_pool(name="singles", bufs=1))
        xload = ctx.enter_context(tc.tile_pool(name="xload", bufs=3))
        stats = ctx.enter_context(tc.tile_pool(name="stats", bufs=6))
        xcs = ctx.enter_context(tc.tile_pool(name="xcs", bufs=3))
        xcsT = ctx.enter_context(tc.tile_pool(name="xcsT", bufs=1))
        wt = ctx.enter_context(tc.tile_pool(name="wt", bufs=9))
        qkT = ctx.enter_context(tc.tile_pool(name="qkT", bufs=1))
        vpool = ctx.enter_context(tc.tile_pool(name="vpool", bufs=1))
        ppool = ctx.enter_context(tc.tile_pool(name="ppool", bufs=3))
        ptp = ctx.enter_context(tc.tile_pool(name="ptp", bufs=8))
        ypool = ctx.enter_context(tc.tile_pool(name="ypool", bufs=1))
        wopool = ctx.enter_context(tc.tile_pool(name="wopool", bufs=MT))
        outp = ctx.enter_context(tc.tile_pool(name="outp", bufs=4))
        zpool = ctx.enter_context(tc.tile_pool(name="zpool", bufs=6))

        psS = ctx.enter_context(tc.tile_pool(name="psS", bufs=2, space="PSUM"))
        psU = ctx.enter_context(tc.tile_pool(name="psU", bufs=2, space="PSUM"))

        # ---- singles ----
        bmask_sb = singles.tile([128, NH * 256], F16)
        nc.sync.dma_start(out=bmask_sb, in_=bmask_d[:, :])
        rel128_sb = singles.tile([128, NH], F32)
        nc.sync.dma_start(out=rel128_sb, in_=rel128_d[:, :])
        bq_sb = singles.tile([128, MT], F32)
        nc.sync.dma_start(out=bq_sb, in_=bq_d[:, :])
        bk_sb = singles.tile([128, MT], F32)
        nc.sync.dma_start(out=bk_sb, in_=bk_d[:, :])
        bv_sb = singles.tile([128, M], F16)
        nc.sync.dma_start(out=bv_sb, in_=bv_d[:, :])
        eps_sb = singles.tile([128, 1], F32)
        nc.vector.memset(eps_sb, LN_EPS)

        # ---- phase 1: layernorm (center+scale) and transpose ----
        xcsT_t = [xcsT.tile([128, T], F16, name=f"xcsT{d}") for d in range(DT)]
        for tt in range(TT):
            xt = xload.tile([128, D], F32)
            nc.sync.dma_start(out=xt, in_=x_d[tt * 128 : (tt + 1) * 128, :])
            st6 = stats.tile([128, 2, 6], F32)
            nc.vector.bn_stats(out=st6[:, 0, :], in_=xt[:, 0:512])
            nc.vector.bn_stats(out=st6[:, 1, :], in_=xt[:, 512:1024])
            mv = stats.tile([128, 2], F32)
            nc.vector.bn_aggr(out=mv, in_=st6)
            sq = stats.tile([128, 1], F32)
            nc.scalar.activation(
                out=sq, in_=mv[:, 1:2], func=mybir.ActivationFunctionType.Sqrt,
                bias=eps_sb[:, :], scale=1.0,
            )
            rstd = stats.tile([128, 1], F32)
            nc.vector.reciprocal(out=rstd, in_=sq)
            xcs_t = xcs.tile([128, D], F16)
            nc.vector.tensor_scalar(
                out=xcs_t, in0=xt, scalar1=mv[:, 0:1], scalar2=rstd,
                op0=mybir.AluOpType.subtract, op1=mybir.AluOpType.mult,
            )
            for d in range(DT):
                nc.sync.dma_start_transpose(
                    out=xcsT_t[d][:, tt * 128 : (tt + 1) * 128],
                    in_=xcs_t[:, d * 128 : (d + 1) * 128],
                )

        # ---- phase 2a: q/k projections -> qT/kT [m, t] fp16 ----
        qkT_t = [qkT.tile([128, T], F16, name=f"qkT{i}") for i in range(2 * MT)]
        for pi, (w_d, b_sb) in enumerate(((wqT_d, bq_sb), (wkT_d, bk_sb))):
            wts = []
            for d in range(DT):
                wtd = wt.tile([128, M], F16)
                nc.sync.dma_start(out=wtd, in_=w_d[d * 128 : (d + 1) * 128, :])
                wts.append(wtd)
            for mt in range(MT):
                for tc4 in range(4):
                    ps = psU.tile([128, 512], F32)
                    for d in range(DT):
                        nc.tensor.matmul(
                            ps,
                            lhsT=wts[d][:, mt * 128 : (mt + 1) * 128],
                            rhs=xcsT_t[d][:, tc4 * 512 : (tc4 + 1) * 512],
                            start=(d == 0), stop=(d == DT - 1),
                        )
                    nc.vector.tensor_scalar(
                        out=qkT_t[pi * MT + mt][:, tc4 * 512 : (tc4 + 1) * 512],
                        in0=ps, scalar1=b_sb[:, mt : mt + 1], scalar2=None,
                        op0=mybir.AluOpType.add,
                    )

        # ---- phase 2b: v projection -> v [t, m] fp16 ----
        v_t = [vpool.tile([128, M], F16, name=f"v{tt}") for tt in range(TT)]
        wvs = []
        for d in range(DT):
            wvd = wt.tile([128, M], F16)
            nc.sync.dma_start(out=wvd, in_=wvT_d[d * 128 : (d + 1) * 128, :])
            wvs.append(wvd)
        for tt in range(TT):
            ps = psU.tile([128, 512], F32)
            for d in range(DT):
                nc.tensor.matmul(
                    ps,
                    lhsT=xcsT_t[d][:, tt * 128 : (tt + 1) * 128],
                    rhs=wvs[d],
                    start=(d == 0), stop=(d == DT - 1),
                )
            nc.vector.tensor_add(out=v_t[tt], in0=ps, in1=bv_sb)

        # ---- phase 3: attention per head ----
        yT_t = [ypool.tile([128, T], F16, name=f"yT{i}") for i in range(MT)]
        for h in range(NH):
            qrow = h // 2
            roff = (h % 2) * 64
            U = None
            for it in range(TT):
                W = (it + 1) * 128
                p_t = ppool.tile([128, T], F16)
                zparts = zpool.tile([128, 2], F32)
                nparts = 1 if W <= 1024 else 2
                bstart = max(0, W - 256)  # log-band (+causal mask) columns
                for pj in range(nparts):
                    off = pj * 1024
                    width = min(W - off, 1024)
                    ps = psS.tile([128, 1024], F32)
                    for so in range(0, width, 512):
                        sw = min(512, width - so)
                        nc.tensor.matmul(
                            ps[:, so : so + sw],
                            lhsT=qkT_t[qrow][roff : roff + 64, it * 128 : (it + 1) * 128],
                            rhs=qkT_t[MT + qrow][roff : roff + 64, off + so : off + so + sw],
                            start=True, stop=True,
                        )
                    # add rel-bias band + causal -inf (fused) before exp so that
                    # the accum_out normalizer is exact
                    b0 = max(off, bstart)
                    b1 = min(off + width, W)
                    if b0 < b1:
                        mcol = h * 256 + b0 - W + 256
                        nc.vector.tensor_add(
                            out=ps[:, b0 - off : b1 - off],
                            in0=ps[:, b0 - off : b1 - off],
                            in1=bmask_sb[:, mcol : mcol + (b1 - b0)],
                        )
                    nc.scalar.activation(
                        out=p_t[:, off : off + width],
                        in_=ps[:, :width],
                        func=mybir.ActivationFunctionType.Exp,
                        bias=rel128_sb[:, h : h + 1],
                        scale=1.0,
                        accum_out=zparts[:, pj : pj + 1],
                    )
                if nparts == 2:
                    z = zpool.tile([128, 1], F32)
                    nc.vector.tensor_add(
                        out=z, in0=zparts[:, 0:1], in1=zparts[:, 1:2]
                    )
                else:
                    z = zparts[:, 0:1]
                rz = zpool.tile([128, 1], F32)
                nc.vector.reciprocal(out=rz, in_=z)
                nc.vector.tensor_scalar(
                    out=p_t[:, 0:W], in0=p_t[:, 0:W], scalar1=rz, scalar2=None,
                    op0=mybir.AluOpType.mult,
                )
                if it % 4 == 0:
                    U = psU.tile([128, 512], F32)
                isl = it % 4
                for jt in range(it + 1):
                    ptile = ptp.tile([128, 128], F16)
                    nc.sync.dma_start_transpose(
                        out=ptile, in_=p_t[:, jt * 128 : (jt + 1) * 128]
                    )
                    nc.tensor.matmul(
                        U[0:64, isl * 128 : (isl + 1) * 128],
                        lhsT=v_t[jt][:, h * 64 : (h + 1) * 64],
                        rhs=ptile,
                        start=(jt == 0), stop=(jt == it),
                    )
                if isl == 3:
                    ci = it // 4
                    nc.vector.tensor_copy(
                        out=yT_t[qrow][roff : roff + 64, ci * 512 : (ci + 1) * 512],
                        in_=U[0:64, :],
                    )

        # ---- phase 4: output projection (partial; host adds residual) ----
        wos = []
        for kt in range(MT):
            wod = wopool.tile([128, D], F16)
            nc.sync.dma_start(out=wod, in_=woT_d[kt * 128 : (kt + 1) * 128, :])
            wos.append(wod)
        for tt in range(TT):
            for oc in range(2):
                ps = psU.tile([128, 512], F32)
                for kt in range(MT):
                    nc.tensor.matmul(
                        ps,
                        lhsT=yT_t[kt][:, tt * 128 : (tt + 1) * 128],
                        rhs=wos[kt][:, oc * 512 : (oc + 1) * 512],
                        start=(kt == 0), stop=(kt == MT - 1),
                    )
                osb = outp.tile([128, 512], F32)
                nc.vector.tensor_copy(out=osb, in_=ps)
                nc.sync.dma_start(
                    out=out_d[tt * 128 : (tt + 1) * 128, oc * 512 : (oc + 1) * 512],
                    in_=osb,
                )

    nc.compile()
    return nc


def _host_prep(inputs):
    """Build the 8 per-core input maps."""
    x = np.asarray(inputs["x"], dtype=np.float32)
    Wq = np.asarray(inputs["Wq"], dtype=np.float32)
    Wk = np.asarray(inputs["Wk"], dtype=np.float32)
    Wv = np.asarray(inputs["Wv"], dtype=np.float32)
    Wo = np.asarray(inputs["Wo"], dtype=np.float32)
    rel = np.asarray(inputs["rel"], dtype=np.float32)
    gamma = np.asarray(inputs["ln_gamma"], dtype=np.float32)
    beta = np.asarray(inputs["ln_beta"], dtype=np.float32)

    sc = 1.0 / math.sqrt(DH)
    ii = np.arange(128)

    half = {}
    for hh in range(2):
        hs = slice(hh * M, (hh + 1) * M)
        Wq_h, Wk_h, Wv_h, Wo_h = Wq[hs], Wk[hs], Wv[hs], Wo[:, hs]
        wqT = np.ascontiguousarray((Wq_h * gamma[None, :] * sc).T).astype(np.float16)
        wkT = np.ascontiguousarray((Wk_h * gamma[None, :]).T).astype(np.float16)
        wvT = np.ascontiguousarray((Wv_h * gamma[None, :]).T).astype(np.float16)
        woT = np.ascontiguousarray(Wo_h.T).astype(np.float16)
        bq = ((Wq_h @ beta) * sc).reshape(MT, 128).T.astype(np.float32)
        bk = (Wk_h @ beta).reshape(MT, 128).T.astype(np.float32)
        bv = np.tile((Wv_h @ beta)[None, :], (128, 1)).astype(np.float16)

        bmask = np.zeros((128, NH * 256), dtype=np.float32)
        rel128 = np.zeros((128, NH), dtype=np.float32)
        di = ii[:, None] - ii[None, :]  # ii - jj
        for h in range(NH):
            g = hh * NH + h
            r128 = rel[g, 128]
            b0 = np.where(di >= 0, rel[g, np.clip(di, 0, 128)] - r128, MASK_NEG)
            b1 = rel[g, np.minimum(128 + di, 128)] - r128
            bmask[:, h * 256 : h * 256 + 128] = b1
            bmask[:, h * 256 + 128 : h * 256 + 256] = b0
            rel128[:, h] = r128
        half[hh] = dict(
            wqT=wqT, wkT=wkT, wvT=wvT, woT=woT,
            bq=np.ascontiguousarray(bq), bk=np.ascontiguousarray(bk), bv=bv,
            bmask=bmask.astype(np.float16), rel128=rel128,
        )

    in_maps = []
    for c in range(NCORES):
        b, hh = c // 2, c % 2
        m = dict(half[hh])
        m["x"] = np.ascontiguousarray(x[b])
        in_maps.append(m)
    return in_maps, x


def kernel(**inputs) -> np.ndarray:
    global _CACHED_NC
    if _CACHED_NC is None:
        _CACHED_NC = build_nc()
    nc = _CACHED_NC
    in_maps, x = _host_prep(inputs)
    res = run_bass_kernel_spmd(nc, in_maps, core_ids=list(range(NCORES)))
    out = np.empty_like(x)
    for b in range(4):
        out[b] = x[b] + res.results[2 * b]["out"] + res.results[2 * b + 1]["out"]
    return out


if __name__ == "__main__":
    rng = np.random.default_rng(0)
    fake = {
        "x": rng.standard_normal((4, T, D), dtype=np.float32),
        "Wq": rng.standard_normal((D, D), dtype=np.float32) / 32,
        "Wk": rng.standard_normal((D, D), dtype=np.float32) / 32,
        "Wv": rng.standard_normal((D, D), dtype=np.float32) / 32,
        "Wo": rng.standard_normal((D, D), dtype=np.float32) / 32,
        "rel": np.tile(np.linspace(0, -2, 129, dtype=np.float32), (16, 1)),
        "ln_gamma": np.ones(D, np.float32),
        "ln_beta": np.zeros(D, np.float32),
    }
    y = kernel(**fake)
    print("ran ok", y.shape, y.dtype)



# revision 5
# speedup vs baseline: 3.2887x; 3.2887x over previous
"""Causal MHSA (pre-LN, relative position bias, residual) on 8 Trainium2 cores.

Sharding: batch (4) x head-half (2) -> 8 cores. Core c handles batch c//2 and
heads (c%2)*8 .. (c%2)*8+8. Host sums the two per-batch partials and adds the
residual.

Key idea vs the previous version: NO DMA transposes anywhere (they cost
~1.3us each of serialized DMA-dispatch time and dominated the runtime).
  - Scores are computed directly in transposed [j, i] layout
    (S^T = K^T Q via lhsT=k-slice, rhs=q-slice), so P^T feeds the AV matmul
    with no transposition of P.
  - The softmax denominator Z[i] comes from a ones-column appended to V:
    row 64 of the U' = V'^T P^T accumulator.  1/Z is broadcast across
    partitions with a gpsimd partition_broadcast.
  - x^T for the projections is produced with PE identity-matmul transposes.

Per-core math layout (T=2048, D=1024, dh=64, NH=8, M=512):
  xcsT[d, t]   = ((x - mu) * rstd)^T            (f16, PE transpose)
  qT/kT[m, t]  = (W~^T as lhsT) @ xcsT          (gamma, 1/sqrt(dh) folded in W~)
  v'[t, h, m'] = xcsT as lhsT @ wvT  (+ ones column m'=64 per head)
  S^T[j, i]    = kT-slice as lhsT @ qT-slice    (per head, 512-wide i-blocks,
                                                 causal: j-tiles <= i-block)
  P^T          = exp(S^T + band)   (band = rel bias - rel[128], additive
                                    -30000 mask above the diagonal; softmax is
                                    invariant to the constant rel[128] shift)
  U'[m', i]    = sum_jt v'[jt] as lhsT @ P^T[jt]   (row 64 accumulates Z)
  yT[m, i]     = U'[0:64] * bcast(1/Z)
  out[t, d]    = yT as lhsT @ woT   (partial; host adds pair + residual)
"""

import math
import sys

sys.path.insert(0, "/opt/trn_rl_repo")

import numpy as np
from contextlib import ExitStack

import concourse.bacc as bacc
import concourse.tile as tile
import concourse.mybir as mybir
from concourse.bass_utils import run_bass_kernel_spmd
from concourse.masks import make_identity

F32 = mybir.dt.float32
F16 = mybir.dt.float16

T = 2048
D = 1024
DH = 64
NH = 8  # heads per core
M = NH * DH  # 512 head-dims per core
TT = T // 128  # 16 token tiles
DT = D // 128  # 8 d-chunks
MT = M // 128  # 4 m-tiles
NCORES = 8
LN_EPS = 1e-5
MASK_NEG = -30000.0

_CACHED_NC = None


def build_nc():
    nc = bacc.Bacc("TRN2", target_bir_lowering=False, debug=False, num_devices=NCORES)

    x_d = nc.dram_tensor("x", [T, D], F32, kind="ExternalInput")
    wqT_d = nc.dram_tensor("wqT", [D, M], F16, kind="ExternalInput")
    wkT_d = nc.dram_tensor("wkT", [D, M], F16, kind="ExternalInput")
    wvT_d = nc.dram_tensor("wvT", [D, M], F16, kind="ExternalInput")
    woT_d = nc.dram_tensor("woT", [M, D], F16, kind="ExternalInput")
    bandT_d = nc.dram_tensor("bandT", [128, NH * 256], F16, kind="ExternalInput")
    bq_d = nc.dram_tensor("bq", [128, MT], F32, kind="ExternalInput")
    bk_d = nc.dram_tensor("bk", [128, MT], F32, kind="ExternalInput")
    bv_d = nc.dram_tensor("bv", [128, NH, DH], F16, kind="ExternalInput")
    out_d = nc.dram_tensor("out", [T, D], F32, kind="ExternalOutput")

    with tile.TileContext(nc) as tc, ExitStack() as ctx:
        singles = ctx.enter_context(tc.tile_pool(name="singles", bufs=1))
        xload = ctx.enter_context(tc.tile_pool(name="xload", bufs=3))
        stats = ctx.enter_context(tc.tile_pool(name="stats", bufs=6))
        xcs = ctx.enter_context(tc.tile_pool(name="xcs", bufs=3))
        xcsT = ctx.enter_context(tc.tile_pool(name="xcsT", bufs=1))
        wt = ctx.enter_context(tc.tile_pool(name="wt", bufs=1))
        qkT = ctx.enter_context(tc.tile_pool(name="qkT", bufs=1))
        vpool = ctx.enter_context(tc.tile_pool(name="vpool", bufs=1))
        pts = ctx.enter_context(tc.tile_pool(name="pts", bufs=4))
        rzp = ctx.enter_context(tc.tile_pool(name="rzp", bufs=2))
        rzbp = ctx.enter_context(tc.tile_pool(name="rzbp", bufs=2))
        ypool = ctx.enter_context(tc.tile_pool(name="ypool", bufs=1))
        outp = ctx.enter_context(tc.tile_pool(name="outp", bufs=4))

        psS = ctx.enter_context(tc.tile_pool(name="psS", bufs=3, space="PSUM"))
        psU = ctx.enter_context(tc.tile_pool(name="psU", bufs=3, space="PSUM"))
        psT = ctx.enter_context(tc.tile_pool(name="psT", bufs=2, space="PSUM"))

        # ---- singles ----
        bandT_sb = singles.tile([128, NH * 256], F16)
        nc.sync.dma_start(out=bandT_sb, in_=bandT_d[:, :])
        bq_sb = singles.tile([128, MT], F32)
        nc.sync.dma_start(out=bq_sb, in_=bq_d[:, :])
        bk_sb = singles.tile([128, MT], F32)
        nc.sync.dma_start(out=bk_sb, in_=bk_d[:, :])
        bv_sb = singles.tile([128, NH, DH], F16)
        nc.sync.dma_start(out=bv_sb, in_=bv_d[:, :, :])
        eps_sb = singles.tile([128, 1], F32)
        nc.vector.memset(eps_sb, LN_EPS)
        ident = singles.tile([128, 128], F16)
        make_identity(nc, ident)

        # weights up front (all live; 32 KB/partition total with wo)
        wqs, wks, wvs = [], [], []
        for pi, (w_d, lst) in enumerate(((wqT_d, wqs), (wkT_d, wks), (wvT_d, wvs))):
            for d in range(DT):
                wtd = wt.tile([128, M], F16, name=f"w{pi}_{d}", tag=f"w{pi}_{d}")
                nc.sync.dma_start(out=wtd, in_=w_d[d * 128 : (d + 1) * 128, :])
                lst.append(wtd)
        wos = []
        for kt in range(MT):
            wod = wt.tile([128, D], F16, name=f"wo{kt}", tag=f"wo_{kt}")
            nc.sync.dma_start(out=wod, in_=woT_d[kt * 128 : (kt + 1) * 128, :])
            wos.append(wod)

        # ---- phase 1: layernorm (center+scale), PE transpose -> xcsT ----
        xcsT_t = [xcsT.tile([128, T], F16, name=f"xcsT{d}") for d in range(DT)]
        for tt in range(TT):
            xt = xload.tile([128, D], F32, tag="xt")
            nc.sync.dma_start(out=xt, in_=x_d[tt * 128 : (tt + 1) * 128, :])
            st6 = stats.tile([128, 2, 6], F32, tag="st6")
            nc.vector.bn_stats(out=st6[:, 0, :], in_=xt[:, 0:512])
            nc.vector.bn_stats(out=st6[:, 1, :], in_=xt[:, 512:1024])
            mv = stats.tile([128, 2], F32, tag="mv")
            nc.vector.bn_aggr(out=mv, in_=st6)
            sq = stats.tile([128, 1], F32, tag="sq")
            nc.scalar.activation(
                out=sq, in_=mv[:, 1:2], func=mybir.ActivationFunctionType.Sqrt,
                bias=eps_sb[:, :], scale=1.0,
            )
            rstd = stats.tile([128, 1], F32, tag="rstd")
            nc.vector.reciprocal(out=rstd, in_=sq)
            xcs_t = xcs.tile([128, D], F16, tag="xcs")
            nc.vector.tensor_scalar(
                out=xcs_t, in0=xt, scalar1=mv[:, 0:1], scalar2=rstd,
                op0=mybir.AluOpType.subtract, op1=mybir.AluOpType.mult,
            )
            for d in range(DT):
                pst = psT.tile([128, 128], F16, tag="pst")
                nc.tensor.transpose(pst, xcs_t[:, d * 128 : (d + 1) * 128], ident)
                dst = xcsT_t[d][:, tt * 128 : (tt + 1) * 128]
                if d % 2 == 0:
                    nc.scalar.copy(out=dst, in_=pst)
                else:
                    nc.vector.tensor_copy(out=dst, in_=pst)

        # ---- phase 2a: q/k projections -> qT/kT [m, t] f16 ----
        qkT_t = [qkT.tile([128, T], F16, name=f"qkT{i}") for i in range(2 * MT)]
        for pi, (wts, b_sb) in enumerate(((wqs, bq_sb), (wks, bk_sb))):
            for mt in range(MT):
                for tc4 in range(4):
                    ps = psS.tile([128, 512], F32, tag="s")
                    for d in range(DT):
                        nc.tensor.matmul(
                            ps,
                            lhsT=wts[d][:, mt * 128 : (mt + 1) * 128],
                            rhs=xcsT_t[d][:, tc4 * 512 : (tc4 + 1) * 512],
                            start=(d == 0), stop=(d == DT - 1),
                        )
                    nc.vector.tensor_scalar(
                        out=qkT_t[pi * MT + mt][:, tc4 * 512 : (tc4 + 1) * 512],
                        in0=ps, scalar1=b_sb[:, mt : mt + 1], scalar2=None,
                        op0=mybir.AluOpType.add,
                    )

        # ---- phase 2b: v projection -> v' [t, h, 65] f16 (ones col at m'=64) --
        v_t = [vpool.tile([128, NH, DH + 1], F16, name=f"v{tt}") for tt in range(TT)]
        for tt in range(TT):
            ps = psS.tile([128, 512], F32, tag="s")
            for d in range(DT):
                nc.tensor.matmul(
                    ps,
                    lhsT=xcsT_t[d][:, tt * 128 : (tt + 1) * 128],
                    rhs=wvs[d],
                    start=(d == 0), stop=(d == DT - 1),
                )
            nc.gpsimd.memset(v_t[tt][:, :, DH : DH + 1], 1.0)
            nc.vector.tensor_add(
                out=v_t[tt][:, :, 0:DH],
                in0=ps.rearrange("p (h d) -> p h d", d=DH),
                in1=bv_sb,
            )

        # ---- phase 3: attention per (head, 512-wide i-block) ----
        yT_t = [ypool.tile([128, T], F16, name=f"yT{i}") for i in range(MT)]
        for h in range(NH):
            qrow = h // 2
            roff = (h % 2) * 64
            kT_ap = qkT_t[MT + qrow]
            qT_ap = qkT_t[qrow]
            for ib in range(4):
                jlast = 4 * ib + 3
                Ups = psU.tile([128, 512], F32, tag="u")
                pt_tiles = {}
                for jt in range(jlast + 1):
                    k = jt - 4 * ib
                    c0 = 128 * k if k > 0 else 0
                    ps = psS.tile([128, 512], F32, tag="s")
                    nc.tensor.matmul(
                        ps[:, c0:512],
                        lhsT=kT_ap[roff : roff + 64, jt * 128 : (jt + 1) * 128],
                        rhs=qT_ap[roff : roff + 64, ib * 512 + c0 : (ib + 1) * 512],
                        start=True, stop=True,
                    )
                    # near-diagonal band: rel bias (minus rel[128]) + causal mask
                    bcol = h * 256
                    if 0 <= k <= 2:
                        nc.vector.tensor_add(
                            out=ps[:, 128 * k : 128 * k + 256],
                            in0=ps[:, 128 * k : 128 * k + 256],
                            in1=bandT_sb[:, bcol : bcol + 256],
                        )
                    elif k == 3:
                        nc.vector.tensor_add(
                            out=ps[:, 384:512],
                            in0=ps[:, 384:512],
                            in1=bandT_sb[:, bcol : bcol + 128],
                        )
                    elif k == -1:
                        nc.vector.tensor_add(
                            out=ps[:, 0:128],
                            in0=ps[:, 0:128],
                            in1=bandT_sb[:, bcol + 128 : bcol + 256],
                        )
                    pt = pts.tile([128, 512], F16, tag="pt")
                    nc.scalar.activation(
                        out=pt[:, c0:512],
                        in_=ps[:, c0:512],
                        func=mybir.ActivationFunctionType.Exp,
                        bias=0.0,
                        scale=1.0,
                    )
                    pt_tiles[jt] = (pt, c0)
                    # software pipeline: AV lags the S^T/exp stream by 2
                    if jt >= 2:
                        avj = jt - 2
                        avpt, avc0 = pt_tiles.pop(avj)
                        nc.tensor.matmul(
                            Ups[0 : DH + 1, avc0:512],
                            lhsT=v_t[avj][:, h, :],
                            rhs=avpt[:, avc0:512],
                            start=(avj == 0), stop=False,
                            skip_group_check=True,
                        )
                for avj in range(max(0, jlast - 1), jlast + 1):
                    avpt, avc0 = pt_tiles.pop(avj)
                    nc.tensor.matmul(
                        Ups[0 : DH + 1, avc0:512],
                        lhsT=v_t[avj][:, h, :],
                        rhs=avpt[:, avc0:512],
                        start=(avj == 0), stop=(avj == jlast),
                        skip_group_check=True,
                    )
                rz = rzp.tile([1, 512], F32, tag="rz")
                nc.vector.reciprocal(out=rz, in_=Ups[DH : DH + 1, :])
                rzb = rzbp.tile([64, 512], F32, tag="rzb")
                nc.gpsimd.partition_broadcast(rzb, rz)
                nc.vector.tensor_mul(
                    out=yT_t[qrow][roff : roff + 64, ib * 512 : (ib + 1) * 512],
                    in0=Ups[0:DH, :],
                    in1=rzb,
                )

        # ---- phase 4: output projection (partial; host adds residual) ----
        for tt in range(TT):
            for oc in range(2):
                ps = psU.tile([128, 512], F32, tag="u")
                for kt in range(MT):
                    nc.tensor.matmul(
                        ps,
                        lhsT=yT_t[kt][:, tt * 128 : (tt + 1) * 128],
                        rhs=wos[kt][:, oc * 512 : (oc + 1) * 512],
                        start=(kt == 0), stop=(kt == MT - 1),
                    )
                osb = outp.tile([128, 512], F32, tag="o")
                if oc == 0:
                    nc.scalar.copy(out=osb, in_=ps)
                else:
                    nc.vector.tensor_copy(out=osb, in_=ps)
                nc.sync.dma_start(
                    out=out_d[tt * 128 : (tt + 1) * 128, oc * 512 : (oc + 1) * 512],
                    in_=osb,
                )

    nc.compile()
    return nc


def _host_prep(inputs):
    """Build the 8 per-core input maps."""
    x = np.asarray(inputs["x"], dtype=np.float32)
    Wq = np.asarray(inputs["Wq"], dtype=np.float32)
    Wk = np.asarray(inputs["Wk"], dtype=np.float32)
    Wv = np.asarray(inputs["Wv"], dtype=np.float32)
    Wo = np.asarray(inputs["Wo"], dtype=np.float32)
    rel = np.asarray(inputs["rel"], dtype=np.float32)
    gamma = np.asarray(inputs["ln_gamma"], dtype=np.float32)
    beta = np.asarray(inputs["ln_beta"], dtype=np.float32)

    sc = 1.0 / math.sqrt(DH)
    ii = np.arange(128)

    half = {}
    for hh in range(2):
        hs = slice(hh * M, (hh + 1) * M)
        Wq_h, Wk_h, Wv_h, Wo_h = Wq[hs], Wk[hs], Wv[hs], Wo[:, hs]
        wqT = np.ascontiguousarray((Wq_h * gamma[None, :] * sc).T).astype(np.float16)
        wkT = np.ascontiguousarray((Wk_h * gamma[None, :]).T).astype(np.float16)
        wvT = np.ascontiguousarray((Wv_h * gamma[None, :]).T).astype(np.float16)
        woT = np.ascontiguousarray(Wo_h.T).astype(np.float16)
        bq = ((Wq_h @ beta) * sc).reshape(MT, 128).T.astype(np.float32)
        bk = (Wk_h @ beta).reshape(MT, 128).T.astype(np.float32)
        bv = np.tile((Wv_h @ beta)[None, :], (128, 1)).reshape(128, NH, DH)

        # transposed band tiles [j, i]: value depends on d0 = i - j
        bandT = np.zeros((128, NH * 256), dtype=np.float32)
        d0 = ii[None, :] - ii[:, None]  # [j, i] = i - j
        for h in range(NH):
            g = hh * NH + h
            r128 = rel[g, 128]
            diag = np.where(d0 >= 0, rel[g, np.clip(d0, 0, 128)] - r128, MASK_NEG)
            offd = rel[g, np.minimum(d0 + 128, 128)] - r128
            bandT[:, h * 256 : h * 256 + 128] = diag
            bandT[:, h * 256 + 128 : h * 256 + 256] = offd
        half[hh] = dict(
            wqT=wqT, wkT=wkT, wvT=wvT, woT=woT,
            bq=np.ascontiguousarray(bq), bk=np.ascontiguousarray(bk),
            bv=bv.astype(np.float16),
            bandT=bandT.astype(np.float16),
        )

    in_maps = []
    for c in range(NCORES):
        b, hh = c // 2, c % 2
        m = dict(half[hh])
        m["x"] = np.ascontiguousarray(x[b])
        in_maps.append(m)
    return in_maps, x


def kernel(**inputs) -> np.ndarray:
    global _CACHED_NC
    if _CACHED_NC is None:
        _CACHED_NC = build_nc()
    nc = _CACHED_NC
    in_maps, x = _host_prep(inputs)
    res = run_bass_kernel_spmd(nc, in_maps, core_ids=list(range(NCORES)))
    out = np.empty_like(x)
    for b in range(4):
        out[b] = x[b] + res.results[2 * b]["out"] + res.results[2 * b + 1]["out"]
    return out


if __name__ == "__main__":
    rng = np.random.default_rng(0)
    fake = {
        "x": rng.standard_normal((4, T, D), dtype=np.float32),
        "Wq": rng.standard_normal((D, D), dtype=np.float32) / 32,
        "Wk": rng.standard_normal((D, D), dtype=np.float32) / 32,
        "Wv": rng.standard_normal((D, D), dtype=np.float32) / 32,
        "Wo": rng.standard_normal((D, D), dtype=np.float32) / 32,
        "rel": np.tile(np.linspace(0, -2, 129, dtype=np.float32), (16, 1)),
        "ln_gamma": np.ones(D, np.float32),
        "ln_beta": np.zeros(D, np.float32),
    }
    y = kernel(**fake)
    print("ran ok", y.shape, y.dtype)
